# revision 12
# baseline (speedup 1.0000x reference)
"""GAU (Gated Attention Unit) Trainium2 kernel, fp8 pipeline.

Full inputs in, full outputs out.  Sharding: data-parallel over batch
(4 batches x 2 cores); within a batch pair each core owns half the
sequence (2048 query rows) and computes k/v for all 4096 rows locally
(no collectives).  Per-core inputs are reordered own-rows-first so the
SPMD program uses uniform addressing; attention is permutation
invariant over the key axis so the reorder is harmless.

Device pipeline per core (fp8 DoubleRow matmuls for 4x PE throughput):
  LN stats via bn_stats/bn_aggr (DVE) -> rsqrt via pow(-0.5) (DVE) ->
  normalize (Act, Identity w/ per-row scale+bias) -> batched DMA
  transpose (bf16) -> cast to fp8 normT8 -> Z/v/gate matmuls in fp8
  DoubleRow -> silu on Act (fused Silu table) -> per 512-row i-block:
  sim matmuls (bf16, qk=128 contraction), A = relu(sim)^2 as a single
  scalar_tensor_tensor (x max 0)*x split DVE/(Act relu + Pool mult),
  A@v in fp8 DoubleRow, vg gate-mult (DVE), out matmul fp8 DoubleRow,
  residual add, DMA out.

Numerics: the attention branch is attenuated by gamma (~0.02) and
1/seq_len, so fp8 e4m3 needs static power-of-2 rescaling to stay in
range: q carries 2^a (folded into gamma0/S on host), A = relu(sim*2^a)^2
carries 2^2a, vg carries 2^cv more, weights carry 64; the final
residual op multiplies by the exact power-of-2 descale.  a and cv are
derived from gamma/seq_len magnitudes on the host.

norm_scale/norm_bias are folded into W_hidden/W_qk on the host.
"""

import os
import sys

import numpy as np

for _p in ("/opt/trn_rl_repo", "/root/.axon_site/_ro/trn_rl_repo"):
    if os.path.isdir(_p) and _p not in sys.path:
        sys.path.insert(0, _p)
        break

import ml_dtypes  # noqa: E402

import concourse.bass as bass  # noqa: E402
import concourse.tile as tile  # noqa: E402
from concourse import mybir  # noqa: E402

AF = mybir.ActivationFunctionType
ALU = mybir.AluOpType
AX = mybir.AxisListType
DT = mybir.dt
PM = mybir.MatmulPerfMode
BF16 = ml_dtypes.bfloat16
F8 = ml_dtypes.float8_e4m3

B, S, D = 4, 4096, 512
H = 1024          # v width == gate width
QK = 128
SO = S // 2       # own rows per core
NCORES = 8
EPS = 1e-5

RT = 32           # row tiles of 128 over S
GT = 4            # LN groups of 8 row-tiles (1024 rows)
FC = D // 128     # feature chunks (4)
HC = H // 128     # hidden chunks (8)
IB = 512          # attention i-block
NBLK = SO // IB   # 4
JT = S // 128     # key chunks (32)
WSC = 64.0        # weight fp8 scale (2^6)


def _build(flags, split=True):
    """Build the SPMD Bass program.  flags = (use_bv, use_bout, use_b1)."""
    use_bv, use_bout, use_b1 = flags
    nc = bass.Bass()

    xa_d = nc.declare_dram_parameter("xa", [S, D], DT.float32, isOutput=False)
    whid_d = nc.declare_dram_parameter("whid", [D, 2 * H], DT.float8e4, isOutput=False)
    wqk_d = nc.declare_dram_parameter("wqk", [D, QK], DT.float8e4, isOutput=False)
    wout_d = nc.declare_dram_parameter("wout", [H, D], DT.float8e4, isOutput=False)
    bqk_d = nc.declare_dram_parameter("bqk", [QK], DT.float32, isOutput=False)
    bg_d = nc.declare_dram_parameter("bg", [H], DT.float32, isOutput=False)
    g0s_d = nc.declare_dram_parameter("g0s", [QK], DT.float32, isOutput=False)
    be0s_d = nc.declare_dram_parameter("be0s", [QK], DT.float32, isOutput=False)
    g1_d = nc.declare_dram_parameter("g1", [QK], DT.float32, isOutput=False)
    be1_d = nc.declare_dram_parameter("be1", [QK], DT.float32, isOutput=False)
    # scalar scales, shipped as 1-elem tensors folded at build time instead?
    # No: they vary with input stats, ship as [2] tensor (svg, desc).
    sc_d = nc.declare_dram_parameter("sc", [2], DT.float32, isOutput=False)
    if use_bv:
        bv_d = nc.declare_dram_parameter("bv", [H], DT.float32, isOutput=False)
    if use_bout:
        bout_d = nc.declare_dram_parameter("bout", [D], DT.float32, isOutput=False)
    out_d = nc.declare_dram_parameter("out", [SO, D], DT.float32, isOutput=True)

    with tile.TileContext(nc) as tc:
        with tc.tile_pool(name="persist", bufs=1) as pp:
            kT = pp.tile([128, S], DT.bfloat16)
            qT = pp.tile([128, SO], DT.bfloat16)
            v8 = pp.tile([128, RT, H], DT.float8e4)
            gT8 = pp.tile([128, HC, SO], DT.float8e4)
            normT8 = pp.tile([128, FC, S], DT.float8e4)
            whid8 = pp.tile([128, FC, 2 * H], DT.float8e4)
            wqk8 = pp.tile([128, FC, QK], DT.float8e4)
            wout8 = pp.tile([128, HC, D], DT.float8e4)
            bqk_sb = pp.tile([128, 1], DT.float32)
            bg_sb = pp.tile([128, HC], DT.float32)
            g0s_sb = pp.tile([128, 1], DT.float32)
            be0s_sb = pp.tile([128, 1], DT.float32)
            g1_sb = pp.tile([128, 1], DT.float32)
            be1_sb = pp.tile([128, 1], DT.float32)
            sc_sb = pp.tile([128, 2], DT.float32)
            nc.sync.dma_start(whid8[:], whid_d[:].rearrange("(f p) h -> p f h", p=128))
            nc.sync.dma_start(wqk8[:], wqk_d[:].rearrange("(f p) q -> p f q", p=128))
            nc.sync.dma_start(wout8[:], wout_d[:].rearrange("(c p) d -> p c d", p=128))
            nc.sync.dma_start(bqk_sb[:], bqk_d[:].unsqueeze(1))
            nc.sync.dma_start(bg_sb[:], bg_d[:].rearrange("(c p) -> p c", p=128))
            nc.sync.dma_start(g0s_sb[:], g0s_d[:].unsqueeze(1))
            nc.sync.dma_start(be0s_sb[:], be0s_d[:].unsqueeze(1))
            nc.sync.dma_start(g1_sb[:], g1_d[:].unsqueeze(1))
            nc.sync.dma_start(be1_sb[:], be1_d[:].unsqueeze(1))
            nc.sync.dma_start(
                sc_sb[:], sc_d[:].unsqueeze(0).partition_broadcast(128)
            )
            if use_bv:
                bv_rep = pp.tile([128, H], DT.float32)
                nc.sync.dma_start(
                    bv_rep[:], bv_d[:].unsqueeze(0).partition_broadcast(128)
                )
            if use_bout:
                bout_rep = pp.tile([128, D], DT.float32)
                nc.sync.dma_start(
                    bout_rep[:], bout_d[:].unsqueeze(0).partition_broadcast(128)
                )

            # ---------- phase 1+2: LN -> transpose -> cast -> Z/v/gate ----
            with (
                tc.tile_pool(name="lnx", bufs=2) as lnx,
                tc.tile_pool(name="lnst", bufs=2) as lnst,
                tc.tile_pool(name="lnb", bufs=3) as lnb,
                tc.tile_pool(name="ntb", bufs=2) as ntb,
                tc.tile_pool(name="zp", bufs=2, space="PSUM") as zp,
                tc.tile_pool(name="zs", bufs=2) as zs,
                tc.tile_pool(name="vp", bufs=2, space="PSUM") as vp,
                tc.tile_pool(name="vt", bufs=2) as vt,
                tc.tile_pool(name="gp", bufs=2, space="PSUM") as gp,
            ):
                for g in range(GT):
                    xg = lnx.tile([128, 8, D], DT.float32, tag="xg")
                    nc.sync.dma_start(
                        xg[:],
                        xa_d[g * 1024 : (g + 1) * 1024, :].rearrange(
                            "(t p) d -> p t d", p=128
                        ),
                    )
                    st6 = lnst.tile([128, 8, 6], DT.float32, tag="st6")
                    mv = lnst.tile([128, 8, 2], DT.float32, tag="mv")
                    veps = lnst.tile([128, 8], DT.float32, tag="veps")
                    nvh = lnst.tile([128, 8], DT.float32, tag="nvh")
                    y2 = lnst.tile([128, 8], DT.float32, tag="y2")
                    h = lnst.tile([128, 8], DT.float32, tag="h")
                    rsv = lnst.tile([128, 8], DT.float32, tag="rsv")
                    nmu = lnst.tile([128, 8], DT.float32, tag="nmu")
                    for t in range(8):
                        nc.vector.bn_stats(st6[:, t, :], xg[:, t, :])
                        nc.vector.bn_aggr(mv[:, t, :], st6[:, t, :])
                    # rsv = rsqrt(var + eps), via Newton (DVE has no rsqrt /
                    # pow).  LN'd unit-gaussian rows have var ~= 1, so the
                    # linear seed y0 = (3 - v)/2 converges in 3 iterations to
                    # ~f32 precision for v in (0, 3).
                    nc.vector.tensor_scalar_add(veps[:], mv[:, :, 1], EPS)
                    nc.vector.tensor_scalar(
                        rsv[:], veps[:], -0.5, 1.5, ALU.mult, ALU.add
                    )
                    nc.vector.tensor_scalar_mul(nvh[:], veps[:], -0.5)
                    for _ in range(3):
                        nc.vector.tensor_mul(y2[:], rsv[:], rsv[:])
                        nc.vector.tensor_mul(h[:], y2[:], nvh[:])
                        nc.vector.tensor_scalar_add(h[:], h[:], 1.5)
                        nc.vector.tensor_mul(rsv[:], rsv[:], h[:])
                    nc.vector.scalar_tensor_tensor(
                        nmu[:], mv[:, :, 0], -1.0, rsv[:], ALU.mult, ALU.mult
                    )
                    ntg = ntb.tile([128, FC, 1024], DT.bfloat16, tag="ntg")
                    for t in range(8):
                        nb = lnb.tile([128, D], DT.bfloat16, tag="nb")
                        nc.gpsimd.tensor_scalar(
                            nb[:], xg[:, t, :],
                            rsv[:, t : t + 1], nmu[:, t : t + 1],
                            ALU.mult, ALU.add,
                        )
                        nc.sync.dma_start(
                            ntg[:, :, t * 128 : (t + 1) * 128], nb[:],
                            transpose=True,
                        )
                    for f in range(FC):
                        nc.vector.tensor_scalar_add(
                            normT8[:, f, g * 1024 : (g + 1) * 1024],
                            ntg[:, f, :], 0.0,
                        )

                    # --- Z chunks for this group's columns ---
                    # beta1 == 0 fast path: fold gamma1 into the q-side scale
                    # (g0s := gamma0*gamma1*sA/S, be0s := beta0*gamma1*sA/S on
                    # the host) so kT is the raw silu output written by the
                    # Act engine directly and only qT needs a DVE scale op.
                    for n in (2 * g, 2 * g + 1):
                        c0 = n * 512
                        ps = zp.tile([128, 512], DT.float32, tag="zp")
                        for f2 in range(0, FC, 2):
                            nc.tensor.matmul(
                                ps[:], wqk8[:, f2 : f2 + 2, :],
                                normT8[:, f2 : f2 + 2, c0 : c0 + 512],
                                start=(f2 == 0), stop=(f2 == FC - 2),
                                perf_mode=PM.DoubleRow,
                            )
                        if use_b1:
                            sil = zs.tile([128, 512], DT.float32, tag="sil")
                            nc.scalar.activation(
                                sil[:], ps[:], AF.Silu,
                                bias=bqk_sb[:, 0:1], scale=1.0 / WSC,
                            )
                            nc.vector.tensor_scalar(
                                kT[:, c0 : c0 + 512], sil[:],
                                g1_sb[:, 0:1], be1_sb[:, 0:1],
                                ALU.mult, ALU.add,
                            )
                            if n < SO // 512:
                                nc.vector.tensor_scalar(
                                    qT[:, c0 : c0 + 512], sil[:],
                                    g0s_sb[:, 0:1], be0s_sb[:, 0:1],
                                    ALU.mult, ALU.add,
                                )
                        else:
                            nc.scalar.activation(
                                kT[:, c0 : c0 + 512], ps[:], AF.Silu,
                                bias=bqk_sb[:, 0:1], scale=1.0 / WSC,
                            )
                            if n < SO // 512:
                                nc.vector.tensor_scalar(
                                    qT[:, c0 : c0 + 512], kT[:, c0 : c0 + 512],
                                    g0s_sb[:, 0:1], be0s_sb[:, 0:1],
                                    ALU.mult, ALU.add,
                                )

                    # --- v row-tiles for this group ---
                    for t in range(8):
                        r = g * 8 + t
                        ps = vp.tile([128, H], DT.float32, tag="vp")
                        for hh in range(2):
                            for f2 in range(0, FC, 2):
                                nc.tensor.matmul(
                                    ps[:, hh * 512 : (hh + 1) * 512],
                                    normT8[:, f2 : f2 + 2, r * 128 : (r + 1) * 128],
                                    whid8[:, f2 : f2 + 2, hh * 512 : (hh + 1) * 512],
                                    start=(f2 == 0), stop=(f2 == FC - 2),
                                    perf_mode=PM.DoubleRow,
                                )
                        if use_bv:
                            tmp = vt.tile([128, H], DT.float32, tag="tmp")
                            nc.vector.scalar_tensor_tensor(
                                tmp[:], ps[:], 1.0 / WSC, bv_rep[:],
                                ALU.mult, ALU.add,
                            )
                            nc.scalar.activation(v8[:, r, :], tmp[:], AF.Silu)
                        else:
                            nc.scalar.activation(
                                v8[:, r, :], ps[:], AF.Silu, scale=1.0 / WSC
                            )

                    # --- gate chunks (own rows only: groups 0,1) ---
                    if g < 2:
                        for ic in (2 * g, 2 * g + 1):
                            i0 = ic * 512
                            for h in range(HC):
                                ps = gp.tile([128, 512], DT.float32, tag="gp")
                                for f2 in range(0, FC, 2):
                                    nc.tensor.matmul(
                                        ps[:],
                                        whid8[:, f2 : f2 + 2,
                                              H + h * 128 : H + (h + 1) * 128],
                                        normT8[:, f2 : f2 + 2, i0 : i0 + 512],
                                        start=(f2 == 0), stop=(f2 == FC - 2),
                                        perf_mode=PM.DoubleRow,
                                    )
                                nc.scalar.activation(
                                    gT8[:, h, i0 : i0 + 512], ps[:], AF.Silu,
                                    bias=bg_sb[:, h : h + 1], scale=1.0 / WSC,
                                )

            # ---------- phase 3: attention + output ----------
            with (
                tc.tile_pool(name="attnA", bufs=1) as pa,
                tc.tile_pool(name="attnR", bufs=3) as pr,
                tc.tile_pool(name="attnVg", bufs=2) as pvg,
                tc.tile_pool(name="attnX", bufs=2) as px,
                tc.tile_pool(name="attnO", bufs=2) as po_sb,
                tc.tile_pool(name="psim", bufs=2, space="PSUM") as psim,
                tc.tile_pool(name="pV", bufs=1, space="PSUM") as pV,
                tc.tile_pool(name="pout", bufs=2, space="PSUM") as pout,
            ):
                A8 = pa.tile([128, JT, IB], DT.float8e4)
                # A = relu(sim)^2 needs two elementwise ops per tile (relu
                # reads PSUM -- only Act/DVE can; square reads SBUF bf16 --
                # Act/DVE/Pool).  Spread tiles over engine-path mixes so no
                # single engine becomes the bottleneck.
                APAT = [
                    "ap", "ap", "dd", "ap", "da", "ap", "dd", "ap",
                    "ap", "dd", "ap", "ap", "dd", "ap", "dd", "ap",
                ]
                for blk in range(NBLK):
                    i0 = blk * IB
                    # simT -> A^T = relu(sim)^2
                    for j in range(JT):
                        ps = psim.tile([128, IB], DT.float32)
                        nc.tensor.matmul(
                            ps[:], kT[:, j * 128 : (j + 1) * 128],
                            qT[:, i0 : i0 + IB], start=True, stop=True,
                        )
                        path = APAT[j % 16]
                        rt = pr.tile([128, IB], DT.bfloat16, tag="rt")
                        if path[0] == "a":
                            nc.scalar.activation(rt[:], ps[:], AF.Relu)
                        else:
                            nc.vector.tensor_scalar_max(rt[:], ps[:], 0.0)
                        if path[1] == "p":
                            nc.gpsimd.tensor_tensor(
                                A8[:, j, :], rt[:], rt[:], ALU.mult
                            )
                        elif path[1] == "d":
                            nc.vector.tensor_mul(A8[:, j, :], rt[:], rt[:])
                        else:
                            nc.scalar.activation(A8[:, j, :], rt[:], AF.Square)
                    # V^T accumulation in fp8 DoubleRow, two h-halves of 512
                    vg = pvg.tile([128, HC, IB], DT.float8e4)
                    for hh in range(2):
                        pvt = [
                            pV.tile(
                                [128, IB], DT.float32,
                                name=f"pvt{q}", tag=f"pvt{q}",
                            )
                            for q in range(4)
                        ]
                        for j2 in range(JT // 2):
                            for hq in range(4):
                                h = hh * 4 + hq
                                nc.tensor.matmul(
                                    pvt[hq][:],
                                    v8[:, 2 * j2 : 2 * j2 + 2,
                                       h * 128 : (h + 1) * 128],
                                    A8[:, 2 * j2 : 2 * j2 + 2, :],
                                    start=(j2 == 0), stop=(j2 == JT // 2 - 1),
                                    perf_mode=PM.DoubleRow,
                                )
                        for hq in range(4):
                            h = hh * 4 + hq
                            nc.vector.scalar_tensor_tensor(
                                vg[:, h, :], pvt[hq][:], sc_sb[:, 0:1],
                                gT8[:, h, i0 : i0 + IB], ALU.mult, ALU.mult,
                            )
                    # out = Vg^T-stationary @ W_out, descale, + x residual
                    xo = px.tile([128, NBLK, D], DT.float32)
                    nc.sync.dma_start(
                        xo[:],
                        xa_d[i0 : i0 + IB, :].rearrange("(c p) d -> p c d", p=128),
                    )
                    ot = po_sb.tile([128, NBLK, D], DT.float32)
                    for ic in range(IB // 128):
                        ps = pout.tile([128, D], DT.float32)
                        for h2 in range(0, HC, 2):
                            nc.tensor.matmul(
                                ps[:], vg[:, h2 : h2 + 2, ic * 128 : (ic + 1) * 128],
                                wout8[:, h2 : h2 + 2, :],
                                start=(h2 == 0), stop=(h2 == HC - 2),
                                perf_mode=PM.DoubleRow,
                            )
                        nc.vector.scalar_tensor_tensor(
                            ot[:, ic, :], ps[:], sc_sb[:, 1:2], xo[:, ic, :],
                            ALU.mult, ALU.add,
                        )
                        if use_bout:
                            nc.vector.tensor_add(
                                ot[:, ic, :], ot[:, ic, :], bout_rep[:]
                            )
                    nc.sync.dma_start(
                        out_d[i0 : i0 + IB, :].rearrange("(c p) d -> p c d", p=128),
                        ot[:],
                    )

    nc.finalize()
    if split:
        _split_waits(nc)
    return nc


# The walrus build in this container supports very few semaphore waits per
# hardware instruction (an Activation with 2 waits or a Drain with 3 fails
# codegen with "Too many sync wait commands").  Tile freely emits
# multi-wait instructions, so hoist all but one wait of each instruction
# into dedicated single-wait EventSemaphore instructions placed immediately
# before it on the same engine queue — semantically identical, just split.
_MAX_WAITS = 1


def _split_waits(nc):
    n_new = 0
    for fn in nc.m.functions:
        for bb in fn.blocks:
            out = []
            changed = False
            for inst in bb.instructions:
                si = inst.sync_info
                if si is not None and len(si.on_wait) > _MAX_WAITS:
                    waits = list(si.on_wait)
                    for w in waits[:-_MAX_WAITS]:
                        es = mybir.InstEventSemaphore(
                            name=f"{inst.name}-w{n_new}", ins=[], outs=[],
                            engine=inst.engine,
                        )
                        es.sync_info = mybir.SyncInfo(on_wait=[w], on_update=[])
                        out.append(es)
                        n_new += 1
                    inst.sync_info = mybir.SyncInfo(
                        on_wait=waits[-_MAX_WAITS:],
                        on_update=list(si.on_update),
                    )
                    changed = True
                out.append(inst)
            if changed:
                bb.instructions = out
    return n_new


_PROGRAM_CACHE = {}


def _get_program(flags):
    if flags not in _PROGRAM_CACHE:
        _PROGRAM_CACHE[flags] = _build(flags)
    return _PROGRAM_CACHE[flags]


def _prep(inputs):
    x = np.ascontiguousarray(np.asarray(inputs["x"], dtype=np.float32))
    scale = np.asarray(inputs["norm_scale"], dtype=np.float32)
    bias = np.asarray(inputs["norm_bias"], dtype=np.float32)
    Wh = np.asarray(inputs["W_hidden"], dtype=np.float32)
    bh = np.asarray(inputs["b_hidden"], dtype=np.float32)
    Wq = np.asarray(inputs["W_qk"], dtype=np.float32)
    bq = np.asarray(inputs["b_qk"], dtype=np.float32)
    gamma = np.asarray(inputs["gamma"], dtype=np.float32)
    beta = np.asarray(inputs["beta"], dtype=np.float32)
    Wo = np.asarray(inputs["W_out"], dtype=np.float32)
    bo = np.asarray(inputs["b_out"], dtype=np.float32)

    # Fold layernorm affine into the following linears.
    Whf = scale[:, None] * Wh
    bhf = bias @ Wh + bh
    Wqf = scale[:, None] * Wq
    bqf = bias @ Wq + bq

    bv = bhf[:H]
    bg = bhf[H:]
    use_bv = bool(np.any(bv != 0.0))
    use_bout = bool(np.any(bo != 0.0))
    use_b1 = bool(np.any(beta[1] != 0.0))

    # fp8 range management: q carries 2^a so sim lands near sigma~0.35,
    # vg carries 2^cv more so vg lands near O(1).  All powers of two, the
    # residual op multiplies by the exact combined descale.
    g0, g1 = gamma[0], gamma[1]
    gg = (g0 * g1).astype(np.float64)
    sig_est = float(np.sqrt(np.sum(gg * gg) * 0.3)) / S
    a = int(np.clip(np.round(np.log2(0.35 / max(sig_est, 1e-30))), 0, 60))
    sA = float(2.0**a)
    EA = (sig_est * sA) ** 2 / 2.0
    V_est = S * EA * 0.5
    cv = int(np.clip(np.round(np.log2(4.0 / max(V_est, 1e-30))), -60, 60))
    svg = float(2.0**cv)
    desc = float(2.0 ** (-2 * a - cv)) / WSC

    if use_b1:
        g0s_h = g0 * (sA / S)
        be0s_h = beta[0] * (sA / S)
    else:
        # beta1 == 0: kT is the raw silu output; fold gamma1 into q's scale
        g0s_h = g0 * g1 * (sA / S)
        be0s_h = beta[0] * g1 * (sA / S)

    common = {
        "whid": (Whf * WSC).astype(F8),
        "wqk": (Wqf * WSC).astype(F8),
        "wout": (Wo * WSC).astype(F8),
        "bqk": np.ascontiguousarray(bqf),
        "bg": np.ascontiguousarray(bg),
        "g0s": np.ascontiguousarray(g0s_h),
        "be0s": np.ascontiguousarray(be0s_h),
        "g1": np.ascontiguousarray(g1),
        "be1": np.ascontiguousarray(beta[1]),
        "sc": np.asarray([svg, desc], dtype=np.float32),
    }
    if use_bv:
        common["bv"] = np.ascontiguousarray(bv)
    if use_bout:
        common["bout"] = np.ascontiguousarray(bo)

    in_maps = []
    for c in range(NCORES):
        b, hlf = divmod(c, 2)
        own = x[b, hlf * SO : (hlf + 1) * SO]
        oth = x[b, (1 - hlf) * SO : (2 - hlf) * SO]
        xa = np.ascontiguousarray(np.concatenate([own, oth], axis=0))
        in_maps.append({**common, "xa": xa})
    return (use_bv, use_bout, use_b1), in_maps


def run_spmd(in_maps, flags, **kw):
    from concourse.bass_utils import run_bass_kernel_spmd

    nc = _get_program(flags)
    return run_bass_kernel_spmd(nc, in_maps, list(range(NCORES)), **kw)


def kernel(**inputs):
    flags, in_maps = _prep(inputs)
    res = run_spmd(in_maps, flags)
    out = np.empty((B, S, D), dtype=np.float32)
    for c in range(NCORES):
        b, hlf = divmod(c, 2)
        out[b, hlf * SO : (hlf + 1) * SO] = res.results[c]["out"]
    return out
